# revision 4
# baseline (speedup 1.0000x reference)
"""Trainium2 Bass kernel for nn_CharEncoder (bi-LSTM char encoder).

Strategy (8 NeuronCores, one SPMD program, per-core data):
  core c: dir = c//4 (0 = left LSTM, 1 = right LSTM), batch slice = c%4 (16 rows).
  Per core: gather embeddings (indirect DMA) -> PE-transpose to feature-major ->
  proj GEMM + tanh -> Wih GEMM (input-gate preactivations) to DRAM scratch ->
  256-step LSTM scan (Whh stationary tiles, bf16 matmuls, fp32 cell state).
Host side only reformats weights (transpose/permute/cast) and slices indices;
all model compute runs on device.
"""
import sys

sys.path.insert(0, "/opt/trn_rl_repo")

import numpy as np
import ml_dtypes

import concourse.bass as bass
import concourse.bacc as bacc
import concourse.tile as tile
import concourse.mybir as mybir
from concourse.bass_utils import run_bass_kernel_spmd
from concourse.masks import make_identity

# Problem constants (hardcoded per harness contract).
VC, VB = 8000, 200000
DC = 100
E, H = 512, 512
B, S = 64, 256
P = 128
BL = B // 4          # local batch per core (4 batch slices x 2 dirs = 8 cores)
T = S * BL           # tokens per core = 4096
NJ = T // P          # 32 token tiles of 128
NT512 = T // 512     # 8 n-tiles of 512 tokens
KC = E // P          # 4 contraction chunks of 128
MC = (4 * H) // P    # 16 gate-row chunks of 128
F = 4 * DC           # 400 input features

DT_BF = mybir.dt.bfloat16
DT_F32 = mybir.dt.float32
NP_BF = ml_dtypes.bfloat16

SCAN_REPS = 1  # >1 only for timing builds (idempotent re-runs)

_CACHE = {}


def _build_program(reps=1):
    key = ("nc", reps)
    if key in _CACHE:
        return _CACHE[key]

    nc = bacc.Bacc("TRN2", target_bir_lowering=False, debug=False, num_devices=8)

    dram = {}

    def din(name, shape, dt):
        dram[name] = nc.dram_tensor(name, shape, dt, kind="ExternalInput").ap()
        return dram[name]

    idxc = din("idxc", [P, NJ], mybir.dt.int32)
    idxb = din("idxb", [P, NJ], mybir.dt.int32)
    ctab = din("ctab", [VC, 2 * DC], DT_F32)      # [char_static | char] cols
    btab = din("btab", [VB, 2 * DC], DT_F32)      # [bichar_static | bichar] cols
    wt = din("wt", [F, E], DT_BF)                 # proj W.T
    pb = din("pb", [P, KC], DT_F32)               # proj bias chunks
    wiht = din("wiht", [E, 4 * H], DT_BF)         # Wih[perm].T
    whht = din("whht", [E, 4 * H], DT_BF)         # Whh[perm].T
    gb = din("gb", [P, MC], DT_F32)               # (bih+bhh)[perm] chunks
    out_ap = nc.dram_tensor("out", [S, 4, P, BL], DT_F32, kind="ExternalOutput").ap()

    with tile.TileContext(nc) as tc:
        with (
            tc.tile_pool(name="const", bufs=1) as cpool,
            tc.tile_pool(name="dram", bufs=1, space="DRAM") as dpool,
        ):
            ident = cpool.tile([P, P], DT_F32)
            make_identity(nc, ident[:])
            idxc_sb = cpool.tile([P, NJ], mybir.dt.int32)
            idxb_sb = cpool.tile([P, NJ], mybir.dt.int32)
            nc.sync.dma_start(out=idxc_sb[:], in_=idxc[:])
            nc.sync.dma_start(out=idxb_sb[:], in_=idxb[:])
            whht_sb = []
            for k in range(KC):
                w = cpool.tile([P, 4 * H], DT_BF, tag=f"whht{k}", name=f"whht{k}")
                nc.sync.dma_start(out=w[:], in_=whht[k * P:(k + 1) * P, :])
                whht_sb.append(w)
            pb_sb = cpool.tile([P, KC], DT_F32)
            gb_sb = cpool.tile([P, MC], DT_F32)
            nc.sync.dma_start(out=pb_sb[:], in_=pb[:])
            nc.sync.dma_start(out=gb_sb[:], in_=gb[:])
            wx_dram = dpool.tile([MC, P, T], DT_F32)

            for rep in range(reps):
                # ---- Phases 1-3 inside their own pool scope (frees SBUF/PSUM for scan)
                with (
                    tc.tile_pool(name="mid", bufs=1) as mpool,
                    tc.tile_pool(name="gath", bufs=4) as gpool,
                    tc.tile_pool(name="pst", bufs=2, space="PSUM") as pst,
                    tc.tile_pool(name="psg", bufs=3, space="PSUM") as psg,
                    tc.tile_pool(name="stage", bufs=4) as spool,
                ):
                    wt_sb = []
                    for k in range(KC):
                        kp = min(P, F - k * P)
                        w = mpool.tile([P, E], DT_BF, tag=f"wt{k}", name=f"wt{k}")
                        nc.sync.dma_start(out=w[:kp, :], in_=wt[k * P:k * P + kp, :])
                        wt_sb.append(w)
                    wiht_sb = []
                    for k in range(KC):
                        w = mpool.tile([P, 4 * H], DT_BF, tag=f"wiht{k}", name=f"wiht{k}")
                        nc.sync.dma_start(out=w[:], in_=wiht[k * P:(k + 1) * P, :])
                        wiht_sb.append(w)

                    # xin^T feature-chunk tiles (bf16) and x^T tiles
                    xinT = [
                        mpool.tile([P, T], DT_BF, tag=f"xinT{k}", name=f"xinT{k}") for k in range(KC)
                    ]
                    xT = [mpool.tile([P, T], DT_BF, tag=f"xT{k}", name=f"xT{k}") for k in range(KC)]

                    # Phase 1: gather + transpose
                    for j in range(NJ):
                        xg = gpool.tile([P, F], DT_F32, tag="xg")
                        nc.gpsimd.indirect_dma_start(
                            out=xg[:, 0:2 * DC], out_offset=None, in_=ctab[:],
                            in_offset=bass.IndirectOffsetOnAxis(
                                ap=idxc_sb[:, j:j + 1], axis=0),
                        )
                        nc.gpsimd.indirect_dma_start(
                            out=xg[:, 2 * DC:F], out_offset=None, in_=btab[:],
                            in_offset=bass.IndirectOffsetOnAxis(
                                ap=idxb_sb[:, j:j + 1], axis=0),
                        )
                        for fc in range(KC):
                            w = min(P, F - fc * P)
                            pt = pst.tile([P, P], DT_F32, tag="pt", space="PSUM")
                            nc.tensor.transpose(
                                out=pt[:w, :], in_=xg[:, fc * P:fc * P + w],
                                identity=ident[:])
                            nc.vector.tensor_copy(
                                out=xinT[fc][:w, j * P:(j + 1) * P], in_=pt[:w, :])

                    # Phase 2: x^T = tanh(wt.T @ xin^T + b)
                    for nt in range(NT512):
                        ns = slice(nt * 512, (nt + 1) * 512)
                        for m in range(KC):
                            ps = psg.tile([P, 512], DT_F32, tag="ps", name="psp", space="PSUM")
                            for k in range(KC):
                                kp = min(P, F - k * P)
                                nc.tensor.matmul(
                                    out=ps[:],
                                    lhsT=wt_sb[k][:kp, m * P:(m + 1) * P],
                                    rhs=xinT[k][:kp, ns],
                                    start=(k == 0), stop=(k == KC - 1),
                                )
                            nc.scalar.activation(
                                out=xT[m][:, ns], in_=ps[:],
                                func=mybir.ActivationFunctionType.Tanh,
                                bias=pb_sb[:, m:m + 1], scale=1.0)

                    # Phase 3: Wx^T = wiht.T @ x^T + gbias -> DRAM (fp32)
                    for nt in range(NT512):
                        ns = slice(nt * 512, (nt + 1) * 512)
                        for m in range(MC):
                            ps = psg.tile([P, 512], DT_F32, tag="ps", name="psw", space="PSUM")
                            for k in range(KC):
                                nc.tensor.matmul(
                                    out=ps[:],
                                    lhsT=wiht_sb[k][:, m * P:(m + 1) * P],
                                    rhs=xT[k][:, ns],
                                    start=(k == 0), stop=(k == KC - 1),
                                )
                            st = spool.tile([P, 512], DT_F32, tag="wxs")
                            nc.scalar.activation(
                                out=st[:], in_=ps[:],
                                func=mybir.ActivationFunctionType.Identity,
                                bias=gb_sb[:, m:m + 1], scale=1.0)
                            nc.sync.dma_start(out=wx_dram[m, :, ns], in_=st[:])

                # ---- Phase 4: LSTM scan
                with (
                    tc.tile_pool(name="scan_ps", bufs=2, space="PSUM") as sps,
                    tc.tile_pool(name="state", bufs=3) as stp,
                    tc.tile_pool(name="ew", bufs=4) as ewp,
                    tc.tile_pool(name="wxp", bufs=4) as wxp,
                ):
                    h_prev = stp.tile([P, KC, BL], DT_BF, tag="h")
                    c_prev = stp.tile([P, KC, BL], DT_F32, tag="c")
                    nc.vector.memset(h_prev[:], 0.0)
                    nc.vector.memset(c_prev[:], 0.0)

                    for t in range(S):
                        wx_t = wxp.tile([P, MC, BL], DT_F32, tag="wx")
                        nc.sync.dma_start(
                            out=wx_t[:],
                            in_=wx_dram[:, :, t * BL:(t + 1) * BL].rearrange(
                                "m p b -> p m b"),
                        )
                        # gate preactivation matmuls, one PSUM bank per block
                        psb = [
                            sps.tile([P, 4, BL], DT_F32, tag=f"ps{b}", name=f"psb{b}", space="PSUM")
                            for b in range(4)
                        ]
                        for m in range(MC):
                            blk, gate = divmod(m, 4)
                            for k in range(KC):
                                nc.tensor.matmul(
                                    out=psb[blk][:, gate, :],
                                    lhsT=whht_sb[k][:, m * P:(m + 1) * P],
                                    rhs=h_prev[:, k, :],
                                    start=(k == 0), stop=(k == KC - 1),
                                )
                        h_new = stp.tile([P, KC, BL], DT_BF, tag="h")
                        c_new = stp.tile([P, KC, BL], DT_F32, tag="c")
                        hout = ewp.tile([P, KC, BL], DT_F32, tag="hout")
                        for b in range(4):
                            pre = ewp.tile([P, 4, BL], DT_F32, tag="pre")
                            nc.vector.tensor_add(
                                out=pre[:], in0=psb[b][:], in1=wx_t[:, 4 * b:4 * b + 4, :])
                            act = ewp.tile([P, 4, BL], DT_F32, tag="act")
                            nc.scalar.activation(
                                out=act[:, 0:2, :], in_=pre[:, 0:2, :],
                                func=mybir.ActivationFunctionType.Sigmoid)
                            nc.scalar.activation(
                                out=act[:, 2:3, :], in_=pre[:, 2:3, :],
                                func=mybir.ActivationFunctionType.Tanh)
                            nc.scalar.activation(
                                out=act[:, 3:4, :], in_=pre[:, 3:4, :],
                                func=mybir.ActivationFunctionType.Sigmoid)
                            t1 = ewp.tile([P, BL], DT_F32, tag="t1")
                            t2 = ewp.tile([P, BL], DT_F32, tag="t2")
                            nc.vector.tensor_mul(
                                out=t1[:], in0=act[:, 1, :], in1=c_prev[:, b, :])
                            nc.vector.tensor_mul(
                                out=t2[:], in0=act[:, 0, :], in1=act[:, 2, :])
                            nc.vector.tensor_add(
                                out=c_new[:, b, :], in0=t1[:], in1=t2[:])
                            tcell = ewp.tile([P, BL], DT_F32, tag="tc")
                            nc.scalar.activation(
                                out=tcell[:], in_=c_new[:, b, :],
                                func=mybir.ActivationFunctionType.Tanh)
                            nc.vector.tensor_mul(
                                out=hout[:, b, :], in0=act[:, 3, :], in1=tcell[:])
                            nc.vector.tensor_copy(
                                out=h_new[:, b, :], in_=hout[:, b, :])
                        nc.sync.dma_start(
                            out=out_ap[t].rearrange("k p b -> p k b"), in_=hout[:])
                        h_prev, c_prev = h_new, c_new

    nc.compile()
    _CACHE[key] = nc
    return nc


def _gate_perm():
    rows = []
    for m in range(MC):
        blk, gate = divmod(m, 4)
        start = gate * H + blk * P
        rows.extend(range(start, start + P))
    return np.array(rows)


def _token_idx(insts_slice):
    # insts_slice [BL, S] -> [P, NJ] token-blocked (token t = s*BL + b)
    tok = np.arange(T)
    vals = insts_slice[tok % BL, tok // BL]        # [T]
    return np.ascontiguousarray(vals.reshape(NJ, P).T.astype(np.int32))


def _make_in_maps(inputs):
    f32 = np.float32
    ctab = np.ascontiguousarray(
        np.concatenate([inputs["char_tab_static"], inputs["char_tab"]], axis=1)
    ).astype(f32)
    btab = np.ascontiguousarray(
        np.concatenate([inputs["bichar_tab_static"], inputs["bichar_tab"]], axis=1)
    ).astype(f32)
    perm = _gate_perm()
    per_dir = []
    for d in range(2):
        sfx = "l" if d == 0 else "r"
        W = np.asarray(inputs[f"W_{sfx}"], f32)
        bvec = np.asarray(inputs[f"b_{sfx}"], f32)
        Wih = np.asarray(inputs[f"Wih_{sfx}"], f32)
        Whh = np.asarray(inputs[f"Whh_{sfx}"], f32)
        bsum = (np.asarray(inputs[f"bih_{sfx}"], f32)
                + np.asarray(inputs[f"bhh_{sfx}"], f32))
        per_dir.append({
            "wt": np.ascontiguousarray(W.T).astype(NP_BF),
            "pb": np.ascontiguousarray(bvec.reshape(KC, P).T).astype(f32),
            "wiht": np.ascontiguousarray(Wih[perm].T).astype(NP_BF),
            "whht": np.ascontiguousarray(Whh[perm].T).astype(NP_BF),
            "gb": np.ascontiguousarray(bsum[perm].reshape(MC, P).T).astype(f32),
        })
    in_maps = []
    for c in range(8):
        d, bs = divmod(c, 4)
        bsl = slice(BL * bs, BL * (bs + 1))
        m = {
            "idxc": _token_idx(np.asarray(inputs["insts_char"])[bsl]),
            "idxb": _token_idx(np.asarray(inputs["insts_bichar_l"])[bsl]),
            "ctab": ctab,
            "btab": btab,
        }
        m.update(per_dir[d])
        in_maps.append(m)
    return in_maps


def kernel(**inputs):
    nc = _build_program(reps=SCAN_REPS)
    in_maps = _make_in_maps(inputs)
    res = run_bass_kernel_spmd(nc, in_maps, core_ids=list(range(8)))
    full = np.zeros((S, B, 2 * H), dtype=np.float32)
    for c in range(8):
        d, bs = divmod(c, 4)
        r = res.results[c]["out"]                  # [S, 4, P, BL]
        r = r.transpose(0, 3, 1, 2).reshape(S, BL, H)
        full[:, BL * bs:BL * (bs + 1), H * d:H * (d + 1)] = r
    return full
